# revision 8
# baseline (speedup 1.0000x reference)
"""Trainium2 Bass kernel for nn_Interpolator: zero-stuff upsample x8 + 128-tap FIR (SAME) + x8 gain.

Polyphase formulation: with m indexing 64-sample rows of x and n = 8*q' + r in [0, 512),
    y[512*m + n] = sum_{k=0}^{78} T4[k, m] * H4[k, n]
where T4[k, m] = x[64*m + k - 7] (zero-padded) and
    H4[k, 8*q'+r] = 8 * h[(7-r) + 8*(k-q')]  for 0 <= k-q' <= 15, else 0.

Per core (8 cores, batch-parallel): 16 signals (8 batch rows x {real, imag}).
Per signal: load x with halo as [128, 271] (partition p = x[256p-7 : 256p+264]),
PE-transpose four 79-column slices into T4 [79, 512] (columns interleaved m = 4p + c),
then 4 matmuls lhsT=T4[:, 128t:128t+128], rhs=H4 [79, 512] -> PSUM [128, 512],
copy to SBUF, DMA out contiguously (partition i of tile t holds y[65536t + 512i : +512]).
"""

import numpy as np

import concourse.bass as bass
import concourse.tile as tile
from concourse import bacc, mybir
from concourse.bass_utils import run_bass_kernel_spmd

B = 64
N = 32768
FACTOR = 8
NOUT = N * FACTOR  # 262144
N_CORES = 8
ROWS_PER_CORE = B // N_CORES  # 8
SIGS = 2 * ROWS_PER_CORE  # 16 signals per core (real rows then imag rows)
K = 79  # contraction window length
XCOLS = 271  # 256 + 15 halo
NPAD = 32784  # 7 leading zeros + N + 8 trailing zeros + 1 spare (host-padded)
TILES = 4  # out tiles per signal, each [128 m-rows, 512 samples]

_F32R = mybir.dt.float32r
_F32 = mybir.dt.float32

_NC_CACHE = {}


def _build_nc():
    nc = bacc.Bacc(
        "TRN2",
        target_bir_lowering=False,
        debug=False,
        enable_asserts=False,
        num_devices=N_CORES,
    )
    x = nc.dram_tensor("x", [SIGS, NPAD], _F32R, kind="ExternalInput")
    h4 = nc.dram_tensor("h4", [K, 512], _F32R, kind="ExternalInput")
    ident = nc.dram_tensor("ident", [128, 128], _F32R, kind="ExternalInput")
    y = nc.dram_tensor("y", [SIGS, NOUT], _F32, kind="ExternalOutput")

    with tile.TileContext(nc) as tc:
        with (
            tc.tile_pool(name="consts", bufs=1) as consts,
            tc.tile_pool(name="xpool", bufs=3) as xpool,
            tc.tile_pool(name="t4pool", bufs=2) as t4pool,
            tc.tile_pool(name="opool", bufs=2) as opool,
            tc.tile_pool(name="pt", bufs=2, space="PSUM") as pt_pool,
            tc.tile_pool(name="po", bufs=2, space="PSUM") as po_pool,
        ):
            h4_sb = consts.tile([K, 512], _F32R)
            nc.sync.dma_start(out=h4_sb, in_=h4.ap())
            ident_sb = consts.tile([128, 128], _F32R)
            nc.sync.dma_start(out=ident_sb, in_=ident.ap())

            for sig in range(SIGS):
                xoff = sig * NPAD
                # partition p holds x_pad[256p : 256p + 271] = x[256p - 7 : 256p + 264]
                X = xpool.tile([128, XCOLS], _F32R)
                nc.sync.dma_start(
                    out=X[:, :],
                    in_=bass.AP(tensor=x, offset=xoff, ap=[[256, 128], [1, XCOLS]]),
                )

                # All 4 transposes write one PSUM bank, one bulk copy out.
                # T4[k, 4p + c] = pt4[k, c, p] = X[p, 64c + k] = x[256p + 64c + k - 7]
                T4 = t4pool.tile([K, 512], _F32R)
                T4i = T4[:, :].rearrange("k (p four) -> k four p", four=4)
                pt4 = pt_pool.tile([K, 4, 128], _F32R)
                for c in range(4):
                    nc.tensor.transpose(
                        pt4[:, c, :], X[:, 64 * c : 64 * c + K], ident_sb
                    )
                if sig % 2 == 0:
                    nc.vector.tensor_copy(out=T4i, in_=pt4[:, :, :])
                else:
                    nc.scalar.copy(out=T4i, in_=pt4[:, :, :])

                out_sb = opool.tile([128, TILES * 512], _F32)
                for half in range(2):
                    po = po_pool.tile([128, 1024], _F32)
                    for s in range(2):
                        t = 2 * half + s
                        nc.tensor.matmul(
                            po[:, 512 * s : 512 * (s + 1)],
                            T4[:, 128 * t : 128 * (t + 1)],
                            h4_sb[:, :],
                            start=True,
                            stop=True,
                        )
                    if half == 0:
                        nc.scalar.copy(
                            out=out_sb[:, 0:1024], in_=po
                        )
                    else:
                        nc.vector.tensor_copy(
                            out=out_sb[:, 1024:2048], in_=po
                        )

                # partition i, free (t, n) -> y[sig, 65536t + 512i + n]
                nc.scalar.dma_start(
                    out=bass.AP(
                        tensor=y,
                        offset=sig * NOUT,
                        ap=[[512, 128], [65536, TILES], [1, 512]],
                    ),
                    in_=out_sb[:, :],
                )

    nc.compile()
    return nc


def _get_nc():
    if "nc" not in _NC_CACHE:
        _NC_CACHE["nc"] = _build_nc()
    return _NC_CACHE["nc"]


def _build_h4(h):
    h4 = np.zeros((K, 512), np.float32)
    qp = np.arange(64)
    for t in range(16):
        for r in range(8):
            h4[qp + t, 8 * qp + r] = FACTOR * h[(7 - r) + 8 * t]
    return h4


def _run(x_real, x_imag, fir_filter, trace=False):
    h4 = _build_h4(np.asarray(fir_filter, np.float32))
    ident = np.eye(128, dtype=np.float32)
    in_maps = []
    for c in range(N_CORES):
        rows = slice(c * ROWS_PER_CORE, (c + 1) * ROWS_PER_CORE)
        shard = np.zeros((SIGS, NPAD), np.float32)
        shard[:ROWS_PER_CORE, 7 : 7 + N] = x_real[rows]
        shard[ROWS_PER_CORE:, 7 : 7 + N] = x_imag[rows]
        in_maps.append({"x": shard, "h4": h4, "ident": ident})
    nc = _get_nc()
    res = run_bass_kernel_spmd(nc, in_maps, core_ids=list(range(N_CORES)), trace=trace)
    out = np.empty((2, B, NOUT), np.float32)
    for c in range(N_CORES):
        yc = res.results[c]["y"]
        rows = slice(c * ROWS_PER_CORE, (c + 1) * ROWS_PER_CORE)
        out[0, rows] = yc[:ROWS_PER_CORE]
        out[1, rows] = yc[ROWS_PER_CORE:]
    return out, res


def kernel(x_real, x_imag, fir_filter, factor):
    assert int(factor) == FACTOR
    x_real = np.asarray(x_real, np.float32)
    x_imag = np.asarray(x_imag, np.float32)
    assert x_real.shape == (B, N) and x_imag.shape == (B, N)
    out, _ = _run(x_real, x_imag, fir_filter)
    return out
